# revision 18
# baseline (speedup 1.0000x reference)
"""Llama attention (B=2, S=2048, H=2048, NH=32, NKV=8, D=64) on 8 trn2 cores.

Sharding: tensor-parallel over heads. Core c owns q-heads [4c, 4c+4) and
kv-head c (GQA groups stay aligned). Each core computes its partial
out_c = attn_c @ wo[:, 256c:256c+256].T over the full batch/sequence in
bf16; the host sums the 8 partials in f32.

Fused streaming schedule: projection of token-group g+1, attention of
chunk g, and out-projection of chunk g-1 interleave in one pass, so the
PE never drains (p-state stays ramped) and the ACT exp stream is the
only hard floor. Off-diagonal AV matmuls run as fp8e4 DoubleRow over
key-tile pairs (2x PE rate); exp carries a -2 bias so e4m3 never
overflows (the bias cancels in the softmax ratio). Engine split: ACT
exp + kv copies; DVE rope muls/psum drains/recip; GPSIMD rope shuffles,
trimask, and the normalization muls (it cannot touch PSUM).
"""

import sys

if "/opt/trn_rl_repo" not in sys.path:
    sys.path.insert(0, "/opt/trn_rl_repo")

import numpy as np
import ml_dtypes

import concourse.bass as bass
import concourse.mybir as mybir
import concourse.tile as tile
from concourse import bacc

bf16 = mybir.dt.bfloat16
f16 = mybir.dt.float16
f32 = mybir.dt.f32 if hasattr(mybir.dt, "f32") else mybir.dt.float32
f8e4 = mybir.dt.float8e4
AF = mybir.ActivationFunctionType
PM = mybir.MatmulPerfMode

B = 2
D = 64
QH = 4                      # q heads per core
SCALE = D ** -0.5
VW = 96                     # vaug strip stride (bf16 and fp8 copies)


def _segments(lo, hi, step=512):
    out = []
    while lo < hi:
        nxt = min(hi, (lo // step + 1) * step)
        out.append((lo, nxt))
        lo = nxt
    return out


def build(Sb=2048, H=2048, QCW=512, av_fp8=True):
    ST = B * Sb
    KT = H // 128           # contraction tiles for projections
    DQ = QH * D             # 256
    NP = QH // 2            # head pairs per core
    NG = ST // QCW          # 8 token groups == attention chunks
    NQC = Sb // QCW         # chunks per batch
    NTT = QCW // 128        # token tiles per chunk
    TPG = QCW // 128        # token tiles per group
    NTB = ST // 128         # total token tiles
    hsegs = _segments(0, H)

    nc = bacc.Bacc(trn_type="TRN2")
    xT_d = nc.dram_tensor("xT", [H, ST], bf16, kind="ExternalInput")
    x8_d = nc.dram_tensor("x8", [128, KT // 2, 2, ST], f8e4,
                          kind="ExternalInput")
    wq8_d = nc.dram_tensor("wq8", [128, KT // 2, 2, DQ], f8e4,
                           kind="ExternalInput")
    wkv_d = nc.dram_tensor("wkvT", [H, 2 * D], bf16, kind="ExternalInput")
    wo_d = nc.dram_tensor("woT", [DQ, H], bf16, kind="ExternalInput")
    cos2_d = nc.dram_tensor("cos2", [128, ST], bf16, kind="ExternalInput")
    sinadj_d = nc.dram_tensor("sinadj", [128, ST], bf16, kind="ExternalInput")
    out_d = nc.dram_tensor("out", [ST, H], bf16, kind="ExternalOutput")
    warm_d = nc.dram_tensor("warmscr", [1, 64], f32, kind="Internal")

    with tile.TileContext(nc) as tc:
        with (
            tc.tile_pool(name="consts", bufs=1) as consts,
            tc.tile_pool(name="resident", bufs=1) as res,
            tc.tile_pool(name="xpool", bufs=26) as xpool,
            tc.tile_pool(name="x8pool", bufs=18) as x8pool,
            tc.tile_pool(name="scratch", bufs=3) as scratch,
            tc.tile_pool(name="etp", bufs=3) as etp,
            tc.tile_pool(name="npool", bufs=3) as npool,
            tc.tile_pool(name="avsp", bufs=5) as avsp,
            tc.tile_pool(name="obuf", bufs=5) as obuf,
            tc.tile_pool(name="sc_ps", bufs=2, space="PSUM") as sc_ps,
            tc.tile_pool(name="acc_ps", bufs=1, space="PSUM") as acc_ps,
            tc.tile_pool(name="work_ps", bufs=2, space="PSUM") as work_ps,
        ):
            dummy = consts.tile([128, 512], bf16, name="dummy")
            nc.vector.memset(dummy[:], 0.0)
            bm2 = consts.tile([128, 1], f32, name="bm2")
            nc.vector.memset(bm2[:], -2.0)
            wbig = consts.tile([97, 128], f16, name="wbig")
            nc.vector.memset(wbig[:], 0.0)
            for rw in (0, 64):
                nc.vector.memset(wbig[rw:rw + 1, 0:64], 1.0)
                nc.vector.memset(wbig[rw + 32:rw + 33, 64:128], 1.0)
            trimask = consts.tile([128, 128], bf16, name="trimask")
            nc.vector.memset(trimask[:], 1.0)
            nc.gpsimd.affine_select(
                out=trimask[:], in_=trimask[:],
                compare_op=mybir.AluOpType.is_ge, fill=0.0,
                base=0, pattern=[[1, 128]], channel_multiplier=-1,
            )

            wkv_t = [res.tile([128, 2 * D], bf16, name=f"wkv{kt}")
                     for kt in range(KT)]
            wq8_t = [res.tile([128, 2 * DQ], f8e4, name=f"wq8_{i}")
                     for i in range(KT // 2)]
            cos2 = res.tile([128, ST], bf16, name="cos2")
            sinadj = res.tile([128, ST], bf16, name="sinadj")
            q2 = [res.tile([128, ST], bf16, name=f"q2_{p}") for p in range(NP)]
            k2 = res.tile([128, ST], bf16, name="k2")
            attnT = [res.tile([128, ST], bf16, name=f"attnT{p}") for p in range(NP)]
            vaugAll = res.tile([128, NTB * VW], bf16, name="vaugAll")
            nc.vector.memset(vaugAll[:], 1.0)
            if av_fp8:
                va8 = res.tile([128, NTB * VW], f8e4, name="va8")
            wo_t = []
            for ki in range(DQ // 128):
                wo_t.append(res.tile([128, H], bf16, name=f"wo{ki}"))

            xts = {}
            x8ts = {}

            def dma_group_x(g):
                t8 = []
                for i in range(KT // 2):
                    x8 = x8pool.tile([128, 2 * QCW], f8e4, name=f"x8_{g}_{i}",
                                    tag="x8")
                    nc.sync.dma_start(
                        x8[:].rearrange("p (t n) -> p t n", t=2),
                        x8_d[:, i, :, g * QCW:(g + 1) * QCW])
                    t8.append(x8)
                x8ts[g] = t8
                tiles = []
                for kt in range(KT):
                    xt = xpool.tile([128, QCW], bf16, name=f"x{g}_{kt}",
                                    tag="xt")
                    nc.sync.dma_start(
                        xt[:], xT_d[kt * 128:(kt + 1) * 128,
                                    g * QCW:(g + 1) * QCW])
                    tiles.append(xt)
                xts[g] = tiles

            # ---------------- projection helpers (one m-burst per call)
            pts = {}

            def proj_burst(g, m):
                pt = work_ps.tile([128, QCW], f32, name=f"pp{g}_{m}",
                                  tag="work")
                if m < NP:
                    for i in range(KT // 2):
                        w3 = wq8_t[i][:].rearrange("p (t f) -> p t f", t=2)
                        nc.tensor.matmul(
                            pt[:], w3[:, :, m * 128:(m + 1) * 128],
                            x8ts[g][i][:].rearrange("p (t n) -> p t n", t=2),
                            start=(i == 0), stop=(i == KT // 2 - 1),
                            perf_mode=PM.DoubleRow)
                else:
                    for kt in range(KT):
                        nc.tensor.matmul(pt[:], wkv_t[kt][:],
                                         xts[g][kt][:],
                                         start=(kt == 0), stop=(kt == KT - 1))
                pts[(g, m)] = pt

            def rope_mats(src_sb, rows, tag, c0):
                sh = scratch.tile([128, QCW], bf16, name=f"sh{tag}", tag="sh")
                for r0 in range(0, rows, 64):
                    nc.vector.tensor_copy(sh[r0:r0 + 32, :],
                                          src_sb[r0 + 32:r0 + 64, :])
                    nc.vector.tensor_copy(sh[r0 + 32:r0 + 64, :],
                                          src_sb[r0:r0 + 32, :])
                t1 = scratch.tile([128, QCW], bf16, name=f"t1{tag}", tag="t1")
                nc.vector.tensor_mul(t1[0:rows, :], src_sb[0:rows, :],
                                     cos2[0:rows, c0:c0 + QCW])
                t2 = scratch.tile([128, QCW], bf16, name=f"t2{tag}", tag="t2")
                nc.vector.tensor_mul(t2[0:rows, :], sh[0:rows, :],
                                     sinadj[0:rows, c0:c0 + QCW])
                return t1, t2

            def drain_q(g, m):
                c0 = g * QCW
                qb = scratch.tile([128, QCW], bf16, name=f"qb{g}{m}",
                                  tag=f"qb{m}")
                nc.vector.tensor_scalar_mul(qb[:], pts[(g, m)][:],
                                            1.0 / 64.0)
                t1, t2 = rope_mats(qb, 128, f"{g}q{m}", c0)
                nc.vector.tensor_add(q2[m][:, c0:c0 + QCW], t1[:], t2[:])

            def drain_kv(g):
                c0 = g * QCW
                pt = pts[(g, NP)]
                vb = scratch.tile([64, QCW], bf16, name=f"vb{g}", tag="vb")
                nc.vector.tensor_copy(vb[:], pt[64:128, :])
                kvb = scratch.tile([128, QCW], bf16, name=f"kvb{g}",
                                   tag="kvb")
                nc.vector.tensor_copy(kvb[0:64, :], pt[0:64, :])
                for j in range(TPG):
                    tb = g * TPG + j
                    nc.sync.dma_start_transpose(
                        vaugAll[:, tb * VW:tb * VW + D],
                        vb[:, j * 128:(j + 1) * 128])
                t1, t2 = rope_mats(kvb, 64, f"{g}k", c0)
                nc.vector.tensor_add(k2[0:64, c0:c0 + QCW],
                                     t1[0:64, :], t2[0:64, :])
                nc.vector.tensor_add(k2[64:128, c0:c0 + QCW],
                                     t1[0:64, :], t2[0:64, :])
                if av_fp8:
                    s0 = g * TPG * VW
                    nc.vector.tensor_copy(va8[:, s0:s0 + TPG * VW],
                                          vaugAll[:, s0:s0 + TPG * VW])

            # ---------------- out-projection units for one chunk
            def make_outproj_units(gprev):
                bb, qq = divmod(gprev, NQC)
                units = []
                for tl in range(NTT):
                    t0 = bb * Sb + qq * QCW + tl * 128
                    ob = obuf.tile([128, H], bf16, name=f"ob{t0}", tag="ob")
                    for hc, (lo, hi) in enumerate(hsegs):
                        def unit(t0=t0, ob=ob, lo=lo, hi=hi, hc=hc):
                            ot = work_ps.tile([128, hi - lo], f32,
                                              name=f"ot{t0}_{hc}", tag="work")
                            for ki in range(NP):
                                nc.tensor.matmul(
                                    ot[:], attnT[ki][:, t0:t0 + 128],
                                    wo_t[ki][:, lo:hi],
                                    start=(ki == 0), stop=(ki == NP - 1))
                            if hc % 2 == 0:
                                nc.vector.tensor_copy(ob[:, lo:hi], ot[:])
                            else:
                                nc.scalar.copy(ob[:, lo:hi], ot[:])
                            if hc == len(hsegs) - 1:
                                nc.sync.dma_start(out_d[t0:t0 + 128, :],
                                                  ob[:])
                        units.append(unit)
                return units

            # ---------------- warmup + prelude
            wm = work_ps.tile([128, 512], f32, name="warm", tag="work")
            for i in range(32):
                nc.tensor.matmul(wm[:], dummy[:, 0:128], dummy[:],
                                 start=True, stop=True)
            wmsb = consts.tile([1, D], f32, name="wmsb")
            nc.vector.tensor_copy(wmsb[:], wm[0:1, 0:D])
            nc.sync.dma_start(warm_d[:], wmsb[:])

            for i in range(KT // 2):
                nc.sync.dma_start(
                    wq8_t[i][:].rearrange("p (t f) -> p t f", t=2),
                    wq8_d[:, i])
                x8 = x8pool.tile([128, 2 * QCW], f8e4, name=f"x8_0_{i}",
                                tag="x8")
                nc.sync.dma_start(x8[:].rearrange("p (t n) -> p t n", t=2),
                                  x8_d[:, i, :, 0:QCW])
                x8ts.setdefault(0, []).append(x8)
            for kt in range(KT):
                nc.sync.dma_start(wkv_t[kt][:],
                                  wkv_d[kt * 128:(kt + 1) * 128, :])
                xt = xpool.tile([128, QCW], bf16, name=f"x0_{kt}", tag="xt")
                nc.sync.dma_start(xt[:], xT_d[kt * 128:(kt + 1) * 128,
                                              0:QCW])
                xts.setdefault(0, []).append(xt)
            dma_group_x(1)
            nc.sync.dma_start(cos2[:], cos2_d[:])
            nc.sync.dma_start(sinadj[:], sinadj_d[:])
            for ki in range(DQ // 128):
                nc.sync.dma_start(wo_t[ki][:],
                                  wo_d[ki * 128:(ki + 1) * 128, :])

            proj_burst(0, 0)
            drain_q(0, 0)
            proj_burst(0, 1)
            drain_q(0, 1)
            proj_burst(0, 2)
            drain_kv(0)

            # ---------------- fused main loop
            prev_chunk = None
            for g in range(NG):
                b, qc = divmod(g, NQC)
                b0 = b * Sb
                q0 = b0 + qc * QCW
                nkt = (qc + 1) * (QCW // 128)
                sums4 = npool.tile([97, QCW], f32, name=f"s4{g}", tag="s4")
                if g < 3:
                    nc.vector.memset(sums4[:], 1.0)
                accs = {(p, h): acc_ps.tile([D + 1, QCW], f32,
                                            name=f"acc{g}{p}{h}",
                                            tag=f"acc{h}")
                        for p in range(NP) for h in range(2)}
                jobs = []

                ots = (make_outproj_units(prev_chunk)
                       if prev_chunk is not None else [])
                filler = []
                if g + 1 < NG:
                    gn = g + 1
                    filler.append(lambda gn=gn: (proj_burst(gn, 0),
                                                 drain_q(gn, 0)))
                    filler.extend(ots[0:5])
                    filler.append(lambda gn=gn: (proj_burst(gn, 1),
                                                 drain_q(gn, 1)))
                    filler.extend(ots[5:10])
                    filler.append(lambda gn=gn: (proj_burst(gn, 2),
                                                 drain_kv(gn)))
                    filler.extend(ots[10:])
                else:
                    filler = ots
                fi = 0
                slot_i = 0
                slots_total = NP * nkt

                def emit_avu(unit):
                    if unit[0] == "pair":
                        _, p, kt0, et8t = unit
                        tb = b * (Sb // 128) + kt0
                        va2 = va8[:].rearrange(
                            "p (t w) -> p t w", t=NTB)[:, tb:tb + 2, 0:D + 1]
                        e4v = et8t[:].rearrange(
                            "p (t h w) -> p t h w", t=2, h=2)
                        for h in range(2):
                            nc.tensor.matmul(
                                accs[(p, h)][:, 0:QCW], va2, e4v[:, :, h],
                                start=(kt0 == 0), stop=False,
                                perf_mode=PM.DoubleRow,
                                skip_group_check=True)
                    else:
                        _, p, kt, w0, et = unit
                        tb = b * (Sb // 128) + kt
                        va = vaugAll[:, tb * VW:tb * VW + D + 1]
                        for h in range(2):
                            nc.tensor.matmul(
                                accs[(p, h)][:, w0:QCW], va,
                                et[:, h * QCW + w0:(h + 1) * QCW],
                                start=(kt == 0), stop=(kt == nkt - 1),
                                skip_group_check=True)
                        if kt == nkt - 1:
                            for h in range(2):
                                row = (p * 2 + h) * 32
                                nc.vector.tensor_copy(
                                    sums4[row:row + 1, :],
                                    accs[(p, h)][D:D + 1, :])
                            for h in range(2):
                                avs = avsp.tile([D, QCW], f32,
                                                name=f"av{g}{p}{h}",
                                                tag="avs")
                                nc.vector.tensor_copy(avs[:],
                                                      accs[(p, h)][0:D, :])
                                jobs.append((p, h, q0, avs))

                avq = []
                prev_sc = None
                for p in range(NP):
                    et8cur = None
                    for kt in range(nkt):
                        r = kt * 128 - qc * QCW
                        w0 = max(0, r)
                        sc = sc_ps.tile([128, 2 * QCW], f32,
                                        name=f"sc{g}{p}{kt}", tag="sc")
                        kcols = slice(b0 + kt * 128, b0 + (kt + 1) * 128)
                        for h in range(2):
                            hr = h * 64
                            nc.tensor.matmul(
                                sc[:, h * QCW + w0:(h + 1) * QCW],
                                k2[hr:hr + 64, kcols],
                                q2[p][hr:hr + 64, q0 + w0:q0 + QCW],
                                start=True, stop=True)
                        sc3 = sc[:].rearrange("p (h w) -> p h w", h=2)
                        if av_fp8 and r < 0:
                            if kt % 2 == 0:
                                et8cur = etp.tile([128, 4 * QCW], f8e4,
                                                  name=f"e8{g}{p}{kt}",
                                                  tag="et8")
                            e4v = et8cur[:].rearrange(
                                "p (t h w) -> p t h w", t=2, h=2)
                            nc.scalar.activation(e4v[:, kt % 2], sc3,
                                                 AF.Exp, scale=SCALE,
                                                 bias=bm2[:])
                            if kt % 2 == 1:
                                avq.append(("pair", p, kt - 1, et8cur))
                        else:
                            et = etp.tile([128, 2 * QCW], bf16,
                                          name=f"et{g}{p}{kt}", tag="et")
                            et3 = et[:].rearrange("p (h w) -> p h w", h=2)
                            nc.scalar.activation(et3[:, :, w0:QCW],
                                                 sc3[:, :, w0:QCW],
                                                 AF.Exp, scale=SCALE,
                                                 bias=bm2[:])
                            if r >= 0:
                                for h in range(2):
                                    o = h * QCW + r
                                    nc.gpsimd.tensor_mul(
                                        et[:, o:o + 128],
                                        et[:, o:o + 128], trimask[:])
                            avq.append(("single", p, kt, w0, et))
                        while len(avq) > 1:
                            emit_avu(avq.pop(0))
                        slot_i += 1
                        pumped = False
                        while fi < len(filler) * slot_i // slots_total:
                            filler[fi]()
                            fi += 1
                            pumped = True
                        if not pumped and prev_sc is not None:
                            # burn into the drained previous score tile:
                            # keeps the PE activity window above the p-state
                            # ramp-down threshold without blocking the
                            # score/exp pipeline
                            for _ in range(2):
                                nc.tensor.matmul(
                                    prev_sc[:, 0:QCW], dummy[0:64, 0:128],
                                    dummy[0:64, :], start=True, stop=True,
                                    skip_group_check=True)
                        prev_sc = sc
                    while avq:
                        emit_avu(avq.pop(0))
                while fi < len(filler):
                    filler[fi]()
                    fi += 1

                # normalization chain
                rec4 = npool.tile([97, QCW], f32, name=f"rec{g}", tag="rec4")
                nc.vector.reciprocal_approx_fast(rec4[:], sums4[:])
                rech4 = npool.tile([97, QCW], f16, name=f"rh{g}", tag="rech4")
                nc.vector.tensor_copy(rech4[:], rec4[:])

                def emit_norm():
                    for base, j0 in ((0, 0), (64, 2)):
                        rb2 = work_ps.tile([128, QCW], f32,
                                           name=f"rb{g}{base}", tag="work")
                        nc.tensor.matmul(rb2[:], wbig[base:base + 33, :],
                                         rech4[base:base + 33, :],
                                         start=True, stop=True)
                        for i, (p, h, qq0, avs) in enumerate(jobs[j0:j0 + 2]):
                            nc.vector.tensor_mul(
                                attnT[p][h * 64:h * 64 + 64, qq0:qq0 + QCW],
                                avs[:], rb2[i * 64:(i + 1) * 64, :])

                emit_norm()
                if g + 2 < NG:
                    dma_group_x(g + 2)
                prev_chunk = g

            for u in make_outproj_units(prev_chunk):
                u()

    nc.finalize()
    return nc


_CACHE = {}


def _get_nc(key, **kw):
    if key not in _CACHE:
        _CACHE[key] = build(**kw)
    return _CACHE[key]


def _prep_inputs(x, cos, sin, wq, wk, wv, wo):
    """Host-side sharding/layout prep. Returns list of 8 per-core in_maps."""
    Bx, S, H = x.shape
    bf = ml_dtypes.bfloat16
    x2d = x.reshape(Bx * S, H)
    xT = np.ascontiguousarray(x2d.T).astype(bf)

    cosT = np.concatenate([cos[b].T for b in range(Bx)], axis=1)   # [64, B*S]
    sinT = np.concatenate([sin[b].T for b in range(Bx)], axis=1)
    cos2 = np.tile(cosT, (2, 1)).astype(bf)
    sadj64 = np.concatenate([-sinT[0:32], sinT[32:64]], axis=0)
    sinadj = np.tile(sadj64, (2, 1)).astype(bf)

    e4 = ml_dtypes.float8_e4m3fn
    KT2 = H // 256
    # folded fp8 activations: x8[p, i, t, s] = x[s, i*256 + t*128 + p]
    x8 = np.ascontiguousarray(
        xT.reshape(KT2, 2, 128, Bx * S).transpose(2, 0, 1, 3)).astype(e4)

    in_maps = []
    for c in range(8):
        wq_c = wq[c * 256:(c + 1) * 256]          # (256, H)
        wk_c = wk[c * 64:(c + 1) * 64]            # (64, H)
        wv_c = wv[c * 64:(c + 1) * 64]
        wq8 = np.ascontiguousarray(
            (wq_c.T * 64.0).reshape(KT2, 2, 128, 256)
            .transpose(2, 0, 1, 3)).astype(e4)
        wkvT = np.concatenate([wk_c.T, wv_c.T], axis=1).astype(bf)
        woT = np.ascontiguousarray(wo[:, c * 256:(c + 1) * 256].T).astype(bf)
        in_maps.append({
            "xT": xT, "x8": x8, "wq8": wq8, "cos2": cos2, "sinadj": sinadj,
            "wkvT": np.ascontiguousarray(wkvT),
            "woT": woT,
        })
    return in_maps


LAST_RESULTS = None


def kernel(x, cos, sin, mask, wq, wk, wv, wo):
    global LAST_RESULTS
    from concourse.bass_utils import run_bass_kernel_spmd

    x = np.asarray(x, dtype=np.float32)
    cos = np.asarray(cos, dtype=np.float32)
    sin = np.asarray(sin, dtype=np.float32)
    wq = np.asarray(wq, dtype=np.float32)
    wk = np.asarray(wk, dtype=np.float32)
    wv = np.asarray(wv, dtype=np.float32)
    wo = np.asarray(wo, dtype=np.float32)

    nc = _get_nc("full")
    in_maps = _prep_inputs(x, cos, sin, wq, wk, wv, wo)
    LAST_RESULTS = run_bass_kernel_spmd(nc, in_maps, core_ids=list(range(8)))
    Bx, S, H = x.shape
    out = np.zeros((Bx * S, H), dtype=np.float32)
    for r in LAST_RESULTS.results:
        out += r["out"].astype(np.float32)
    return out.reshape(Bx, S, H)


# revision 19
# speedup vs baseline: 1.0817x; 1.0817x over previous
"""Llama attention (B=2, S=2048, H=2048, NH=32, NKV=8, D=64) on 8 trn2 cores.

Sharding: tensor-parallel over heads. Core c owns q-heads [4c, 4c+4) and
kv-head c (GQA groups stay aligned). Each core computes its partial
out_c = attn_c @ wo[:, 256c:256c+256].T over the full batch/sequence in
bf16; the host sums the 8 partials in f32.

Fused streaming schedule: projection of token-group g+1, attention of
chunk g, and out-projection of chunk g-1 interleave in one pass, so the
PE never drains (p-state stays ramped) and the ACT exp stream is the
only hard floor. Off-diagonal AV matmuls run as fp8e4 DoubleRow over
key-tile pairs (2x PE rate); exp carries a -2 bias so e4m3 never
overflows (the bias cancels in the softmax ratio). Engine split: ACT
exp + kv copies; DVE rope muls/psum drains/recip; GPSIMD rope shuffles,
trimask, and the normalization muls (it cannot touch PSUM).
"""

import sys

if "/opt/trn_rl_repo" not in sys.path:
    sys.path.insert(0, "/opt/trn_rl_repo")

import numpy as np
import ml_dtypes

import concourse.bass as bass
import concourse.mybir as mybir
import concourse.tile as tile
from concourse import bacc

bf16 = mybir.dt.bfloat16
f16 = mybir.dt.float16
f32 = mybir.dt.f32 if hasattr(mybir.dt, "f32") else mybir.dt.float32
f8e4 = mybir.dt.float8e4
AF = mybir.ActivationFunctionType
PM = mybir.MatmulPerfMode

B = 2
D = 64
QH = 4                      # q heads per core
SCALE = D ** -0.5
VW = 96                     # vaug strip stride (bf16 and fp8 copies)


def _segments(lo, hi, step=512):
    out = []
    while lo < hi:
        nxt = min(hi, (lo // step + 1) * step)
        out.append((lo, nxt))
        lo = nxt
    return out


def build(Sb=2048, H=2048, QCW=512, av_fp8=True):
    ST = B * Sb
    KT = H // 128           # contraction tiles for projections
    DQ = QH * D             # 256
    NP = QH // 2            # head pairs per core
    NG = ST // QCW          # 8 token groups == attention chunks
    NQC = Sb // QCW         # chunks per batch
    NTT = QCW // 128        # token tiles per chunk
    TPG = QCW // 128        # token tiles per group
    NTB = ST // 128         # total token tiles
    hsegs = _segments(0, H)

    nc = bacc.Bacc(trn_type="TRN2")
    xT_d = nc.dram_tensor("xT", [H, ST], bf16, kind="ExternalInput")
    wqkv_d = nc.dram_tensor("wqkvT", [H, DQ + 2 * D], bf16, kind="ExternalInput")
    wo_d = nc.dram_tensor("woT", [DQ, H], bf16, kind="ExternalInput")
    cos2_d = nc.dram_tensor("cos2", [128, ST], bf16, kind="ExternalInput")
    sinadj_d = nc.dram_tensor("sinadj", [128, ST], bf16, kind="ExternalInput")
    out_d = nc.dram_tensor("out", [ST, H], bf16, kind="ExternalOutput")
    warm_d = nc.dram_tensor("warmscr", [1, 64], f32, kind="Internal")

    with tile.TileContext(nc) as tc:
        with (
            tc.tile_pool(name="consts", bufs=1) as consts,
            tc.tile_pool(name="resident", bufs=1) as res,
            tc.tile_pool(name="xpool", bufs=34) as xpool,
            tc.tile_pool(name="scratch", bufs=3) as scratch,
            tc.tile_pool(name="etp", bufs=3) as etp,
            tc.tile_pool(name="npool", bufs=3) as npool,
            tc.tile_pool(name="avsp", bufs=5) as avsp,
            tc.tile_pool(name="obuf", bufs=5) as obuf,
            tc.tile_pool(name="sc_ps", bufs=2, space="PSUM") as sc_ps,
            tc.tile_pool(name="acc_ps", bufs=1, space="PSUM") as acc_ps,
            tc.tile_pool(name="work_ps", bufs=2, space="PSUM") as work_ps,
        ):
            dummy = consts.tile([128, 512], bf16, name="dummy")
            nc.vector.memset(dummy[:], 0.0)
            bm2 = consts.tile([128, 1], f32, name="bm2")
            nc.vector.memset(bm2[:], -2.0)
            wbig = consts.tile([97, 128], f16, name="wbig")
            nc.vector.memset(wbig[:], 0.0)
            for rw in (0, 64):
                nc.vector.memset(wbig[rw:rw + 1, 0:64], 1.0)
                nc.vector.memset(wbig[rw + 32:rw + 33, 64:128], 1.0)
            trimask = consts.tile([128, 128], bf16, name="trimask")
            nc.vector.memset(trimask[:], 1.0)
            nc.gpsimd.affine_select(
                out=trimask[:], in_=trimask[:],
                compare_op=mybir.AluOpType.is_ge, fill=0.0,
                base=0, pattern=[[1, 128]], channel_multiplier=-1,
            )

            wqkv_t = [res.tile([128, DQ + 2 * D], bf16, name=f"wqkv{kt}")
                      for kt in range(KT)]
            cos2 = res.tile([128, ST], bf16, name="cos2")
            sinadj = res.tile([128, ST], bf16, name="sinadj")
            q2 = [res.tile([128, ST], bf16, name=f"q2_{p}") for p in range(NP)]
            k2 = res.tile([128, ST], bf16, name="k2")
            attnT = [res.tile([128, ST], bf16, name=f"attnT{p}") for p in range(NP)]
            vaugAll = res.tile([128, NTB * VW], bf16, name="vaugAll")
            nc.vector.memset(vaugAll[:], 1.0)
            if av_fp8:
                va8 = res.tile([128, NTB * VW], f8e4, name="va8")
            wo_t = []
            for ki in range(DQ // 128):
                wo_t.append(res.tile([128, H], bf16, name=f"wo{ki}"))

            xts = {}

            def dma_group_x(g):
                tiles = []
                for kt in range(KT):
                    xt = xpool.tile([128, QCW], bf16, name=f"x{g}_{kt}",
                                    tag="xt")
                    nc.sync.dma_start(
                        xt[:], xT_d[kt * 128:(kt + 1) * 128,
                                    g * QCW:(g + 1) * QCW])
                    tiles.append(xt)
                xts[g] = tiles

            # ---------------- projection helpers (one m-burst per call)
            pts = {}

            def proj_burst(g, m):
                pt = work_ps.tile([128, QCW], f32, name=f"pp{g}_{m}",
                                  tag="work")
                for kt in range(KT):
                    nc.tensor.matmul(pt[:],
                                     wqkv_t[kt][:, m * 128:(m + 1) * 128],
                                     xts[g][kt][:],
                                     start=(kt == 0), stop=(kt == KT - 1))
                pts[(g, m)] = pt

            def rope_mats(src_sb, rows, tag, c0):
                sh = scratch.tile([128, QCW], bf16, name=f"sh{tag}", tag="sh")
                for r0 in range(0, rows, 64):
                    nc.vector.tensor_copy(sh[r0:r0 + 32, :],
                                          src_sb[r0 + 32:r0 + 64, :])
                    nc.vector.tensor_copy(sh[r0 + 32:r0 + 64, :],
                                          src_sb[r0:r0 + 32, :])
                t1 = scratch.tile([128, QCW], bf16, name=f"t1{tag}", tag="t1")
                nc.vector.tensor_mul(t1[0:rows, :], src_sb[0:rows, :],
                                     cos2[0:rows, c0:c0 + QCW])
                t2 = scratch.tile([128, QCW], bf16, name=f"t2{tag}", tag="t2")
                nc.vector.tensor_mul(t2[0:rows, :], sh[0:rows, :],
                                     sinadj[0:rows, c0:c0 + QCW])
                return t1, t2

            def drain_q(g, m):
                c0 = g * QCW
                qb = scratch.tile([128, QCW], bf16, name=f"qb{g}{m}",
                                  tag=f"qb{m}")
                nc.vector.tensor_copy(qb[:], pts[(g, m)][:])
                t1, t2 = rope_mats(qb, 128, f"{g}q{m}", c0)
                nc.vector.tensor_add(q2[m][:, c0:c0 + QCW], t1[:], t2[:])

            def drain_kv(g):
                c0 = g * QCW
                pt = pts[(g, NP)]
                vb = scratch.tile([64, QCW], bf16, name=f"vb{g}", tag="vb")
                nc.vector.tensor_copy(vb[:], pt[64:128, :])
                kvb = scratch.tile([128, QCW], bf16, name=f"kvb{g}",
                                   tag="kvb")
                nc.vector.tensor_copy(kvb[0:64, :], pt[0:64, :])
                for j in range(TPG):
                    tb = g * TPG + j
                    nc.sync.dma_start_transpose(
                        vaugAll[:, tb * VW:tb * VW + D],
                        vb[:, j * 128:(j + 1) * 128])
                t1, t2 = rope_mats(kvb, 64, f"{g}k", c0)
                nc.vector.tensor_add(k2[0:64, c0:c0 + QCW],
                                     t1[0:64, :], t2[0:64, :])
                nc.vector.tensor_add(k2[64:128, c0:c0 + QCW],
                                     t1[0:64, :], t2[0:64, :])
                if av_fp8:
                    s0 = g * TPG * VW
                    nc.vector.tensor_copy(va8[:, s0:s0 + TPG * VW],
                                          vaugAll[:, s0:s0 + TPG * VW])

            # ---------------- out-projection units for one chunk
            def make_outproj_units(gprev):
                bb, qq = divmod(gprev, NQC)
                units = []
                for tl in range(NTT):
                    t0 = bb * Sb + qq * QCW + tl * 128
                    ob = obuf.tile([128, H], bf16, name=f"ob{t0}", tag="ob")
                    for hc, (lo, hi) in enumerate(hsegs):
                        def unit(t0=t0, ob=ob, lo=lo, hi=hi, hc=hc):
                            ot = work_ps.tile([128, hi - lo], f32,
                                              name=f"ot{t0}_{hc}", tag="work")
                            for ki in range(NP):
                                nc.tensor.matmul(
                                    ot[:], attnT[ki][:, t0:t0 + 128],
                                    wo_t[ki][:, lo:hi],
                                    start=(ki == 0), stop=(ki == NP - 1))
                            if hc % 2 == 0:
                                nc.vector.tensor_copy(ob[:, lo:hi], ot[:])
                            else:
                                nc.scalar.copy(ob[:, lo:hi], ot[:])
                            if hc == len(hsegs) - 1:
                                nc.sync.dma_start(out_d[t0:t0 + 128, :],
                                                  ob[:])
                        units.append(unit)
                return units

            # ---------------- warmup + prelude
            wm = work_ps.tile([128, 512], f32, name="warm", tag="work")
            for i in range(32):
                nc.tensor.matmul(wm[:], dummy[:, 0:128], dummy[:],
                                 start=True, stop=True)
            wmsb = consts.tile([1, D], f32, name="wmsb")
            nc.vector.tensor_copy(wmsb[:], wm[0:1, 0:D])
            nc.sync.dma_start(warm_d[:], wmsb[:])

            for kt in range(KT):
                nc.sync.dma_start(wqkv_t[kt][:],
                                  wqkv_d[kt * 128:(kt + 1) * 128, :])
                xt = xpool.tile([128, QCW], bf16, name=f"x0_{kt}", tag="xt")
                nc.sync.dma_start(xt[:], xT_d[kt * 128:(kt + 1) * 128,
                                              0:QCW])
                xts.setdefault(0, []).append(xt)
            dma_group_x(1)
            nc.sync.dma_start(cos2[:], cos2_d[:])
            nc.sync.dma_start(sinadj[:], sinadj_d[:])
            for ki in range(DQ // 128):
                nc.sync.dma_start(wo_t[ki][:],
                                  wo_d[ki * 128:(ki + 1) * 128, :])

            proj_burst(0, 0)
            drain_q(0, 0)
            proj_burst(0, 1)
            drain_q(0, 1)
            proj_burst(0, 2)
            drain_kv(0)

            # ---------------- fused main loop
            prev_chunk = None
            for g in range(NG):
                b, qc = divmod(g, NQC)
                b0 = b * Sb
                q0 = b0 + qc * QCW
                nkt = (qc + 1) * (QCW // 128)
                sums4 = npool.tile([97, QCW], f32, name=f"s4{g}", tag="s4")
                if g < 3:
                    nc.vector.memset(sums4[:], 1.0)
                accs = {(p, h): acc_ps.tile([D + 1, QCW], f32,
                                            name=f"acc{g}{p}{h}",
                                            tag=f"acc{h}")
                        for p in range(NP) for h in range(2)}
                jobs = []

                filler = (make_outproj_units(prev_chunk)
                          if prev_chunk is not None else [])
                fi = 0
                per_slot = -(-len(filler) // (NP * nkt)) if filler else 0

                def emit_avu(unit):
                    if unit[0] == "pair":
                        _, p, kt0, et8t = unit
                        tb = b * (Sb // 128) + kt0
                        va2 = va8[:].rearrange(
                            "p (t w) -> p t w", t=NTB)[:, tb:tb + 2, 0:D + 1]
                        e4v = et8t[:].rearrange(
                            "p (t h w) -> p t h w", t=2, h=2)
                        for h in range(2):
                            nc.tensor.matmul(
                                accs[(p, h)][:, 0:QCW], va2, e4v[:, :, h],
                                start=(kt0 == 0), stop=False,
                                perf_mode=PM.DoubleRow,
                                skip_group_check=True)
                    else:
                        _, p, kt, w0, et = unit
                        tb = b * (Sb // 128) + kt
                        va = vaugAll[:, tb * VW:tb * VW + D + 1]
                        for h in range(2):
                            nc.tensor.matmul(
                                accs[(p, h)][:, w0:QCW], va,
                                et[:, h * QCW + w0:(h + 1) * QCW],
                                start=(kt == 0), stop=(kt == nkt - 1),
                                skip_group_check=True)
                        if kt == nkt - 1:
                            for h in range(2):
                                row = (p * 2 + h) * 32
                                nc.vector.tensor_copy(
                                    sums4[row:row + 1, :],
                                    accs[(p, h)][D:D + 1, :])
                            for h in range(2):
                                avs = avsp.tile([D, QCW], f32,
                                                name=f"av{g}{p}{h}",
                                                tag="avs")
                                nc.vector.tensor_copy(avs[:],
                                                      accs[(p, h)][0:D, :])
                                jobs.append((p, h, q0, avs))

                avq = []
                for p in range(NP):
                    et8cur = None
                    for kt in range(nkt):
                        r = kt * 128 - qc * QCW
                        w0 = max(0, r)
                        sc = sc_ps.tile([128, 2 * QCW], f32,
                                        name=f"sc{g}{p}{kt}", tag="sc")
                        kcols = slice(b0 + kt * 128, b0 + (kt + 1) * 128)
                        for h in range(2):
                            hr = h * 64
                            nc.tensor.matmul(
                                sc[:, h * QCW + w0:(h + 1) * QCW],
                                k2[hr:hr + 64, kcols],
                                q2[p][hr:hr + 64, q0 + w0:q0 + QCW],
                                start=True, stop=True)
                        sc3 = sc[:].rearrange("p (h w) -> p h w", h=2)
                        if av_fp8 and r < 0:
                            if kt % 2 == 0:
                                et8cur = etp.tile([128, 4 * QCW], f8e4,
                                                  name=f"e8{g}{p}{kt}",
                                                  tag="et8")
                            e4v = et8cur[:].rearrange(
                                "p (t h w) -> p t h w", t=2, h=2)
                            nc.scalar.activation(e4v[:, kt % 2], sc3,
                                                 AF.Exp, scale=SCALE,
                                                 bias=bm2[:])
                            if kt % 2 == 1:
                                avq.append(("pair", p, kt - 1, et8cur))
                        else:
                            et = etp.tile([128, 2 * QCW], bf16,
                                          name=f"et{g}{p}{kt}", tag="et")
                            et3 = et[:].rearrange("p (h w) -> p h w", h=2)
                            nc.scalar.activation(et3[:, :, w0:QCW],
                                                 sc3[:, :, w0:QCW],
                                                 AF.Exp, scale=SCALE,
                                                 bias=bm2[:])
                            if r >= 0:
                                for h in range(2):
                                    o = h * QCW + r
                                    nc.gpsimd.tensor_mul(
                                        et[:, o:o + 128],
                                        et[:, o:o + 128], trimask[:])
                            avq.append(("single", p, kt, w0, et))
                        while len(avq) > 1:
                            emit_avu(avq.pop(0))
                        for _ in range(per_slot):
                            if fi < len(filler):
                                filler[fi]()
                                fi += 1

                    while avq:
                        emit_avu(avq.pop(0))
                while fi < len(filler):
                    filler[fi]()
                    fi += 1

                # normalization chain
                rec4 = npool.tile([97, QCW], f32, name=f"rec{g}", tag="rec4")
                nc.vector.reciprocal_approx_fast(rec4[:], sums4[:])
                rech4 = npool.tile([97, QCW], f16, name=f"rh{g}", tag="rech4")
                nc.vector.tensor_copy(rech4[:], rec4[:])

                def emit_norm():
                    for base, j0 in ((0, 0), (64, 2)):
                        rb2 = work_ps.tile([128, QCW], f32,
                                           name=f"rb{g}{base}", tag="work")
                        nc.tensor.matmul(rb2[:], wbig[base:base + 33, :],
                                         rech4[base:base + 33, :],
                                         start=True, stop=True)
                        for i, (p, h, qq0, avs) in enumerate(jobs[j0:j0 + 2]):
                            nc.vector.tensor_mul(
                                attnT[p][h * 64:h * 64 + 64, qq0:qq0 + QCW],
                                avs[:], rb2[i * 64:(i + 1) * 64, :])

                if g + 1 < NG:
                    proj_burst(g + 1, 0)
                    drain_q(g + 1, 0)
                    proj_burst(g + 1, 1)
                    drain_q(g + 1, 1)
                    proj_burst(g + 1, 2)
                    emit_norm()
                    drain_kv(g + 1)
                    if g + 2 < NG:
                        dma_group_x(g + 2)
                else:
                    emit_norm()
                prev_chunk = g

            for u in make_outproj_units(prev_chunk):
                u()

    nc.finalize()
    return nc


_CACHE = {}


def _get_nc(key, **kw):
    if key not in _CACHE:
        _CACHE[key] = build(**kw)
    return _CACHE[key]


def _prep_inputs(x, cos, sin, wq, wk, wv, wo):
    """Host-side sharding/layout prep. Returns list of 8 per-core in_maps."""
    Bx, S, H = x.shape
    bf = ml_dtypes.bfloat16
    x2d = x.reshape(Bx * S, H)
    xT = np.ascontiguousarray(x2d.T).astype(bf)

    cosT = np.concatenate([cos[b].T for b in range(Bx)], axis=1)   # [64, B*S]
    sinT = np.concatenate([sin[b].T for b in range(Bx)], axis=1)
    cos2 = np.tile(cosT, (2, 1)).astype(bf)
    sadj64 = np.concatenate([-sinT[0:32], sinT[32:64]], axis=0)
    sinadj = np.tile(sadj64, (2, 1)).astype(bf)

    in_maps = []
    for c in range(8):
        wq_c = wq[c * 256:(c + 1) * 256]          # (256, H)
        wk_c = wk[c * 64:(c + 1) * 64]            # (64, H)
        wv_c = wv[c * 64:(c + 1) * 64]
        wqkvT = np.concatenate([wq_c.T, wk_c.T, wv_c.T], axis=1).astype(bf)
        woT = np.ascontiguousarray(wo[:, c * 256:(c + 1) * 256].T).astype(bf)
        in_maps.append({
            "xT": xT, "cos2": cos2, "sinadj": sinadj,
            "wqkvT": np.ascontiguousarray(wqkvT),
            "woT": woT,
        })
    return in_maps


LAST_RESULTS = None


def kernel(x, cos, sin, mask, wq, wk, wv, wo):
    global LAST_RESULTS
    from concourse.bass_utils import run_bass_kernel_spmd

    x = np.asarray(x, dtype=np.float32)
    cos = np.asarray(cos, dtype=np.float32)
    sin = np.asarray(sin, dtype=np.float32)
    wq = np.asarray(wq, dtype=np.float32)
    wk = np.asarray(wk, dtype=np.float32)
    wv = np.asarray(wv, dtype=np.float32)
    wo = np.asarray(wo, dtype=np.float32)

    nc = _get_nc("full")
    in_maps = _prep_inputs(x, cos, sin, wq, wk, wv, wo)
    LAST_RESULTS = run_bass_kernel_spmd(nc, in_maps, core_ids=list(range(8)))
    Bx, S, H = x.shape
    out = np.zeros((Bx * S, H), dtype=np.float32)
    for r in LAST_RESULTS.results:
        out += r["out"].astype(np.float32)
    return out.reshape(Bx, S, H)
